# revision 32
# baseline (speedup 1.0000x reference)
"""Capacity-routed MoE layer for Trainium2, expert-parallel across 8 NeuronCores.

Reference semantics (nn_MoELayer): router picks top-2 experts per token; primary
assignment is capacity-limited (cap = N/E = 512, first-come in token order);
overflow tokens try their second choice; still-dropped tokens go through a
fallback self-FFN. Each core runs its own expert's FFN on the <=512 tokens
routed to it plus an F-shard of the fallback FFN (partials summed on host).

Router: core c computes fp32 logits for its 512-token chunk in [token, expert]
layout (tiny PE matmuls, top-2 via free-axis reduce), packs mask1||mask2 as a
[16, 512] bf16 tile, AllGathers. Post-gather tile G is [128, 512] with row
r = c*16 + m (m<8: mask1[e=m]; m>=8: mask2[e=m-8]) so the global capacity scans
run on the free axis at the same cost as a [64, 512] layout; all cross-row
reductions go through host-built constant matmuls that select only valid rows.

Dispatch: one multi-offset indirect scatter builds the slot->token map
(own-expert slots [0,512), fallback [512,640), other experts dropped via
bounds_check), five column DMAs load it back as gather offsets, and one
multi-offset indirect gather pulls all 640 token rows.

DMA queue split: routing-critical DMAs go on nc.sync (SP); constants and
weight streams on nc.scalar (ACT) with explicit prefetch distance so the
SP queue head never blocks FFN weight flow.
"""

import numpy as np

B, T, H, F, E, TOPK = 4, 1024, 1024, 4096, 8, 2
N = B * T              # 4096 tokens
CAP = N // E           # 512 per-expert capacity
FBC = 128              # fallback slot capacity
NSLOT = CAP + FBC + 2  # own slots + fallback + 2 trash rows
NCORES = 8
NCH = N // NCORES      # 512-token router chunk per core
FSH = F // NCORES      # 512-wide fallback F-shard per core
W1PRE = 16             # w1 tiles prefetched ahead
W2PRE = 4              # w2 tiles prefetched ahead

_CACHE = {}
_PHASES = 99
_NOWEIGHTS = False


def _build(debug=False):
    import concourse.bass as bass
    import concourse.mybir as mybir
    import concourse.tile as tile
    from concourse import bacc
    from concourse.masks import make_identity

    dt = mybir.dt

    nc = bacc.Bacc("TRN2", target_bir_lowering=False, debug=False,
                   num_devices=NCORES)

    # ---- inputs ----
    xTc = nc.dram_tensor("xTc", [H, NCH], dt.float32, kind="ExternalInput")
    xN = nc.dram_tensor("xN", [N, H], dt.bfloat16, kind="ExternalInput")
    rwT = nc.dram_tensor("rwT", [H, E], dt.float32, kind="ExternalInput")
    rbrow = nc.dram_tensor("rbrow", [1, E], dt.float32, kind="ExternalInput")
    ones1 = nc.dram_tensor("ones1", [1, 128], dt.float32, kind="ExternalInput")
    T1 = nc.dram_tensor("T1", [128, 128], dt.float16, kind="ExternalInput")
    E1 = nc.dram_tensor("E1", [128, 8], dt.float16, kind="ExternalInput")
    BU = nc.dram_tensor("BU", [8, 128], dt.float32, kind="ExternalInput")
    A1 = nc.dram_tensor("A1", [128, 8], dt.bfloat16, kind="ExternalInput")
    OV = nc.dram_tensor("OV", [8, 128], dt.bfloat16, kind="ExternalInput")
    AD1 = nc.dram_tensor("AD1", [128, 8], dt.float16, kind="ExternalInput")
    AD2 = nc.dram_tensor("AD2", [128, 8], dt.float16, kind="ExternalInput")
    A2 = nc.dram_tensor("A2", [128, 8], dt.bfloat16, kind="ExternalInput")
    E2 = nc.dram_tensor("E2", [128, 8], dt.float32, kind="ExternalInput")
    TL8 = nc.dram_tensor("TL8", [8, 8], dt.float16, kind="ExternalInput")
    on8 = nc.dram_tensor("on8", [8, 1], dt.float16, kind="ExternalInput")
    bm1 = nc.dram_tensor("bm1", [128, 1], dt.float32, kind="ExternalInput")
    IOTA = nc.dram_tensor("IOTA", [128, 642], dt.float16, kind="ExternalInput")
    TOKC = nc.dram_tensor("TOKC", [128, 64], dt.float16, kind="ExternalInput")
    w1c = nc.dram_tensor("w1c", [F // 128, 128, H // 128, 128], dt.bfloat16,
                         kind="ExternalInput")
    b1c = nc.dram_tensor("b1c", [128, F // 128], dt.float32, kind="ExternalInput")
    w2c = nc.dram_tensor("w2c", [H // 128, 128, F // 128, 128], dt.bfloat16,
                         kind="ExternalInput")
    b2c = nc.dram_tensor("b2c", [128, H // 128], dt.float32, kind="ExternalInput")
    sw1c = nc.dram_tensor("sw1c", [H, FSH], dt.bfloat16, kind="ExternalInput")
    sb1c = nc.dram_tensor("sb1c", [128, FSH // 128], dt.float32, kind="ExternalInput")
    sw2c = nc.dram_tensor("sw2c", [FSH, H], dt.bfloat16, kind="ExternalInput")
    sb2c = nc.dram_tensor("sb2c", [128, H // 128], dt.float32, kind="ExternalInput")

    # ---- outputs ----
    yT = nc.dram_tensor("yT", [H, CAP], dt.float32, kind="ExternalOutput")
    fbT = nc.dram_tensor("fbT", [H, FBC], dt.float32, kind="ExternalOutput")
    idxo = nc.dram_tensor("idxo", [CAP + FBC], dt.int32, kind="ExternalOutput")
    cnt = nc.dram_tensor("cnt", [E + 1, 1], dt.float32, kind="ExternalOutput")

    dbg = {}
    if debug:
        for nm in ("dbg_G", "dbg_scf1", "dbg_keep1", "dbg_ohs", "dbg_take2",
                   "dbg_destf"):
            shp = [8, 512] if nm == "dbg_destf" else [128, 512]
            dbg[nm] = nc.dram_tensor(nm, shp, dt.float32, kind="ExternalOutput")

    with tile.TileContext(nc) as tc:
        _emit(nc, tc, bass, mybir, make_identity, {**locals(), **dbg})
    nc.compile()
    return nc


def _tap(nc, t, name, tile_ap):
    if name in t:
        nc.sync.dma_start(t[name][:], tile_ap)


def _emit(nc, tc, bass, mybir, make_identity, t):
    from contextlib import ExitStack
    dt = mybir.dt
    Alu = mybir.AluOpType
    Act = mybir.ActivationFunctionType
    from concourse.tile_rust import add_dep_helper

    with ExitStack() as ctx:
        const = ctx.enter_context(tc.tile_pool(name="const", bufs=1))
        wpool = ctx.enter_context(tc.tile_pool(name="wpool", bufs=1))
        w1s = ctx.enter_context(tc.tile_pool(name="w1s", bufs=W1PRE))
        w2s = ctx.enter_context(tc.tile_pool(name="w2s", bufs=W2PRE))
        rt = ctx.enter_context(tc.tile_pool(name="rt", bufs=1))
        dr = ctx.enter_context(tc.tile_pool(name="dr", bufs=1, space="DRAM"))
        gat = ctx.enter_context(tc.tile_pool(name="gat", bufs=1))
        outp = ctx.enter_context(tc.tile_pool(name="outp", bufs=2))
        ps_r = ctx.enter_context(tc.tile_pool(name="ps_r", bufs=2, space="PSUM"))
        ps_i = ctx.enter_context(tc.tile_pool(name="ps_i", bufs=1, space="PSUM"))
        ps_t = ctx.enter_context(tc.tile_pool(name="ps_t", bufs=2, space="PSUM"))
        ps_m = ctx.enter_context(tc.tile_pool(name="ps_m", bufs=2, space="PSUM"))

        f32, f16, bf16, i32 = dt.float32, dt.float16, dt.bfloat16, dt.int32

        # ---------- critical input first: x^T chunk for the router ----------
        # 4 token-block DMAs so the first router matmuls start ~4us earlier
        xt = rt.tile([128, 8, NCH], f32)
        xtc_r = t["xTc"][:].rearrange("(k p) i -> p k i", p=128)
        for tb in range(4):
            nc.sync.dma_start(xt[:, :, tb * 128:(tb + 1) * 128],
                              xtc_r[:, :, tb * 128:(tb + 1) * 128])

        # ---------- constants (SP queue; no deps, tiny transfers) ----------
        def cdma(shape, dty, name, rearr=None):
            tl = const.tile(shape, dty, name=f"{name}_sb")
            src = t[name][:]
            if rearr:
                src = src.rearrange(*rearr[0], **rearr[1])
            nc.sync.dma_start(tl[:], src)
            return tl

        rwT_sb = const.tile([128, 8, E], f32)
        nc.sync.dma_start(rwT_sb[:], t["rwT"][:].rearrange("(k p) e -> p k e", p=128))
        rbrow_sb = cdma([1, E], f32, "rbrow")
        ones1_sb = cdma([1, 128], f32, "ones1")
        T1_sb = cdma([128, 128], f16, "T1")
        E1_sb = cdma([128, 8], f16, "E1")
        BU_sb = cdma([8, 128], f32, "BU")
        A1_sb = cdma([128, 8], bf16, "A1")
        OV_sb = cdma([8, 128], bf16, "OV")
        AD1_sb = cdma([128, 8], f16, "AD1")
        AD2_sb = cdma([128, 8], f16, "AD2")
        A2_sb = cdma([128, 8], bf16, "A2")
        E2_sb = cdma([128, 8], f32, "E2")
        TL8_sb = cdma([8, 8], f16, "TL8")
        on8_sb = cdma([8, 1], f16, "on8")
        bm1_sb = cdma([128, 1], f32, "bm1")
        iota_sb = cdma([128, 642], f16, "IOTA")
        tokc_sb = cdma([128, 64], f16, "TOKC")
        ident = const.tile([128, 128], f32)
        make_identity(nc, ident[:])
        identb = const.tile([128, 128], bf16)
        make_identity(nc, identb[:])
        identh = const.tile([128, 128], f16)
        make_identity(nc, identh[:])
        zz = const.tile([128, 1], f32)
        nc.vector.memset(zz[:], 0.0)
        zzh = const.tile([128, 1], bf16)
        nc.vector.memset(zzh[:], 0.0)

        # ---------- phase 1: router, [token, expert] layout ----------
        mk4 = rt.tile([128, 4, 16], f32)
        ptA = ps_t.tile([16, NCH], f32, tag="tps")
        for tb in range(4):
            ps_lg = ps_r.tile([128, E], f32, tag="rps")
            for k8 in range(8):
                nc.tensor.matmul(ps_lg[:], lhsT=xt[:, k8, tb * 128:(tb + 1) * 128],
                                 rhs=rwT_sb[:, k8, :], start=(k8 == 0), stop=False)
            nc.tensor.matmul(ps_lg[:], lhsT=ones1_sb[:],
                             rhs=rbrow_sb[:], start=False, stop=True)
            mx = rt.tile([128, 1], f32, tag="mx")
            nc.vector.tensor_reduce(out=mx[:], in_=ps_lg[:],
                                    axis=mybir.AxisListType.X, op=Alu.max)
            nc.vector.tensor_scalar(out=mk4[:, tb, 0:8], in0=ps_lg[:],
                                    scalar1=mx[:, :1], scalar2=None, op0=Alu.is_ge)
            lg2 = rt.tile([128, E], f32, tag="lg2")
            nc.vector.scalar_tensor_tensor(out=lg2[:], in0=mk4[:, tb, 0:8],
                                           scalar=-1e30, in1=ps_lg[:],
                                           op0=Alu.mult, op1=Alu.add)
            mx2 = rt.tile([128, 1], f32, tag="mx2")
            nc.vector.tensor_reduce(out=mx2[:], in_=lg2[:],
                                    axis=mybir.AxisListType.X, op=Alu.max)
            nc.vector.tensor_scalar(out=mk4[:, tb, 8:16], in0=lg2[:],
                                    scalar1=mx2[:, :1], scalar2=None, op0=Alu.is_ge)
            nc.tensor.transpose(ptA[:, tb * 128:(tb + 1) * 128], mk4[:, tb, :],
                                ident[:])

        agin = rt.tile([16, NCH], bf16)
        nc.vector.tensor_copy(agin[:], ptA[:])
        ag_ib = dr.tile([16, NCH], bf16, tag="ag_ib")
        ag_ob = dr.tile([8, 16, NCH], bf16, tag="ag_ob")
        wr_ib = nc.sync.dma_start(ag_ib[:], agin[:])

        # Weight streams fill the AllGather window; emitted after the agin
        # write so the agin transfer isn't queued behind 23us of weights.
        if _NOWEIGHTS:
            return
        sw1_sb = wpool.tile([128, 8, FSH], bf16)
        nc.sync.dma_start(sw1_sb[:], t["sw1c"][:].rearrange("(k p) f -> p k f", p=128))
        sw2_sb = wpool.tile([128, 4, H], bf16)
        nc.sync.dma_start(sw2_sb[:], t["sw2c"][:].rearrange("(k p) h -> p k h", p=128))
        w1t = [None] * (F // 128)
        for m in range(W1PRE):
            w1tile = w1s.tile([128, 8, 128], bf16, tag="w1t", name=f"w1t{m}")
            nc.sync.dma_start(w1tile[:], t["w1c"][m])
            w1t[m] = w1tile
        w2t = [None] * (H // 128)
        for m in range(W2PRE):
            w2tile = w2s.tile([128, F // 128, 128], bf16, tag="w2t", name=f"w2t{m}")
            nc.sync.dma_start(w2tile[:], t["w2c"][m])
            w2t[m] = w2tile
        b1_sb = cdma([128, F // 128], f32, "b1c")
        b2_sb = cdma([128, H // 128], f32, "b2c")
        sb1_sb = cdma([128, FSH // 128], f32, "sb1c")
        sb2_sb = cdma([128, H // 128], f32, "sb2c")

        coll = nc.gpsimd.collective_compute(
            "AllGather", Alu.bypass, replica_groups=[list(range(NCORES))],
            ins=[ag_ib.opt()], outs=[ag_ob.opt()])
        # Tile's shadow-memory tracking misses collective in/out ordering;
        # pin it with explicit sync edges (baseline-proven pattern).
        add_dep_helper(coll.ins, wr_ib.ins, sync=True,
                       reason="collective waits input write")
        # Readback on the DVE queue: the SP queue head must not block on the
        # collective while weight tiles still need to issue.
        G = rt.tile([128, 512], bf16)
        rd = nc.scalar.dma_start(G[:], ag_ob[:].rearrange("c m i -> (c m) i"))
        add_dep_helper(rd.ins, coll.ins, sync=True,
                       reason="read waits collective completion")
        _tap(nc, t, "dbg_G", G[:])
        if _PHASES < 2:
            return

        # ---------- phase 2: primary capacity assignment ----------
        zbc = zzh[:, :1].to_broadcast([128, 512])
        scanA = rt.tile([128, 512], f16)
        nc.vector.tensor_tensor_scan(out=scanA[:], data0=G[:], data1=zbc,
                                     initial=0.0, op0=Alu.add, op1=Alu.add)
        off1 = ps_r.tile([128, 1], f32, tag="rps")
        nc.tensor.matmul(off1[:], lhsT=T1_sb[:], rhs=scanA[:, 511:512],
                         start=True, stop=True)
        tote = ps_r.tile([8, 1], f32, tag="rps")
        nc.tensor.matmul(tote[:], lhsT=E1_sb[:], rhs=scanA[:, 511:512],
                         start=True, stop=True)
        used = rt.tile([8, 1], f32)
        nc.vector.tensor_scalar(out=used[:], in0=tote[:], scalar1=float(CAP),
                                scalar2=None, op0=Alu.min)
        o1bm = rt.tile([128, 1], f32)
        nc.vector.tensor_tensor(out=o1bm[:], in0=bm1_sb[:], in1=off1[:],
                                op=Alu.add)
        kc = rt.tile([128, 512], bf16)
        nc.vector.tensor_scalar(out=kc[:], in0=scanA[:], scalar1=off1[:, :1],
                                scalar2=float(CAP) + 0.5, op0=Alu.add,
                                op1=Alu.is_lt)
        keep1 = rt.tile([128, 512], bf16)
        nc.vector.tensor_tensor(out=keep1[:], in0=kc[:], in1=G[:], op=Alu.mult)
        # d1a on ACT (parallel to the DVE chain): scanA + (off1 + base - 1)
        d1a = rt.tile([128, 512], f16)
        nc.scalar.activation(d1a[:], scanA[:], Act.Identity, bias=o1bm[:, :1])
        used128 = ps_r.tile([128, 1], f32, tag="rps")
        nc.tensor.matmul(used128[:], lhsT=BU_sb[:], rhs=used[:], start=True,
                         stop=True)
        # consume used128 immediately into per-partition scalars
        u512 = rt.tile([128, 1], f32)
        nc.vector.tensor_scalar(out=u512[:], in0=used128[:], scalar1=-1.0,
                                scalar2=float(CAP) + 0.5, op0=Alu.mult,
                                op1=Alu.add)
        ubm = rt.tile([128, 1], f32)
        nc.vector.tensor_tensor(out=ubm[:], in0=bm1_sb[:], in1=used128[:],
                                op=Alu.add)

        # ---------- phase 3: second-choice assignment ----------
        kept8 = ps_r.tile([8, 512], f32, tag="rps")
        nc.tensor.matmul(kept8[:], lhsT=A1_sb[:], rhs=keep1[:], start=True,
                         stop=True)
        ovf8 = rt.tile([8, 512], bf16)
        nc.vector.tensor_scalar(out=ovf8[:], in0=kept8[:], scalar1=-1.0,
                                scalar2=1.0, op0=Alu.mult, op1=Alu.add)
        ovfb = ps_r.tile([128, 512], f32, tag="rps")
        nc.tensor.matmul(ovfb[:], lhsT=OV_sb[:], rhs=ovf8[:], start=True,
                         stop=True)
        ohs = rt.tile([128, 512], bf16)
        nc.vector.tensor_tensor(out=ohs[:], in0=G[:], in1=ovfb[:], op=Alu.mult)
        _tap(nc, t, "dbg_ohs", ohs[:])
        scanB = rt.tile([128, 512], f16)
        nc.vector.tensor_tensor_scan(out=scanB[:], data0=ohs[:], data1=zbc,
                                     initial=0.0, op0=Alu.add, op1=Alu.add)
        offB = ps_r.tile([128, 1], f32, tag="rps")
        nc.tensor.matmul(offB[:], lhsT=T1_sb[:], rhs=scanB[:, 511:512],
                         start=True, stop=True)
        oBu = rt.tile([128, 1], f32)
        nc.vector.scalar_tensor_tensor(out=oBu[:], in0=u512[:], scalar=-1.0,
                                       in1=offB[:], op0=Alu.mult, op1=Alu.add)
        oBubm = rt.tile([128, 1], f32)
        nc.vector.tensor_tensor(out=oBubm[:], in0=ubm[:], in1=offB[:],
                                op=Alu.add)
        cmp2 = rt.tile([128, 512], bf16)
        nc.vector.tensor_scalar(out=cmp2[:], in0=scanB[:], scalar1=oBu[:, :1],
                                scalar2=0.0, op0=Alu.add, op1=Alu.is_lt)
        take2 = rt.tile([128, 512], bf16)
        nc.vector.tensor_tensor(out=take2[:], in0=cmp2[:], in1=ohs[:],
                                op=Alu.mult)
        d2a = rt.tile([128, 512], f16)
        nc.scalar.activation(d2a[:], scanB[:], Act.Identity, bias=oBubm[:, :1])
        _tap(nc, t, "dbg_take2", take2[:])

        # ---------- phase 4: dispatch destinations (own-expert first) ----
        t2tok = ps_r.tile([8, 512], f32, tag="rps")
        nc.tensor.matmul(t2tok[:], lhsT=A2_sb[:], rhs=take2[:], start=True,
                         stop=True)
        drop8 = rt.tile([8, 512], bf16)
        nc.vector.tensor_tensor(out=drop8[:], in0=ovf8[:], in1=t2tok[:],
                                op=Alu.subtract)
        dest2 = rt.tile([128, 512], f16)
        nc.vector.tensor_tensor(out=dest2[:], in0=d2a[:], in1=take2[:],
                                op=Alu.mult)
        dest1 = rt.tile([128, 512], f16)
        nc.vector.tensor_tensor(out=dest1[:], in0=d1a[:], in1=keep1[:],
                                op=Alu.mult)
        dest8 = ps_r.tile([8, 512], f32, tag="rps")
        nc.tensor.matmul(dest8[:], lhsT=AD1_sb[:], rhs=dest1[:], start=True,
                         stop=False)
        nc.tensor.matmul(dest8[:], lhsT=AD2_sb[:], rhs=dest2[:], start=False,
                         stop=True)
        # own-expert dests; dropped tokens pushed to 3000 so they match nothing
        destfa = rt.tile([8, 512], f32)
        nc.vector.scalar_tensor_tensor(out=destfa[:], in0=drop8[:],
                                       scalar=3000.0, in1=dest8[:],
                                       op0=Alu.mult, op1=Alu.add)

        if _PHASES < 5:
            return
        # ---------- phase 5a: invert own-expert token->slot, gather ----------
        # HW indirect DMA only honors one offset per partition per op, so a
        # scatter-based slot map costs 32 serialized SWDGE ops.  Instead the
        # permutation is inverted on PE/DVE: per 128-token block b,
        # D_b[p, s] = (dest(token_b(p)) == s) built by one is_equal against an
        # iota row; icol[s] = sum_b D_b^T @ [128*t_hi, p] accumulated in fp32
        # PSUM (both rhs columns are f16-exact; their sum is the token id).
        # Interleaved accumulation groups share icps' 2KB PSUM zero-region, so
        # memset once and keep start=False throughout (start=True would re-mark
        # the region and wipe other groups' partials).
        ptDa = ps_t.tile([128, 32], f32, tag="tps")
        for ib in range(4):
            nc.tensor.transpose(ptDa[:, ib * 8:(ib + 1) * 8],
                                destfa[:, ib * 128:(ib + 1) * 128], ident[0:8, 0:8])
        icps = ps_i.tile([128, 10], f32)
        nc.vector.memset(icps[:], 0.0)
        for c in range(8):
            for ib in range(4):
                b = c * 4 + ib
                Db = rt.tile([128, 512], f16, tag="Db", name=f"Db{b}", bufs=6)
                nc.vector.tensor_scalar(out=Db[:], in0=iota_sb[:, 0:512],
                                        scalar1=ptDa[:, ib * 8 + c:ib * 8 + c + 1],
                                        scalar2=None, op0=Alu.is_equal)
                for s in range(4):
                    nc.tensor.matmul(icps[:, 2 * s:2 * s + 2],
                                     lhsT=Db[:, s * 128:(s + 1) * 128],
                                     rhs=tokc_sb[:, 2 * b:2 * b + 2],
                                     start=False, stop=(b == 31),
                                     skip_group_check=True)
        icolT = gat.tile([128, 5], i32)
        with nc.allow_low_precision(reason="token ids are exact ints < 4096"):
            for s in range(4):
                nc.vector.tensor_reduce(out=icolT[:, s:s + 1],
                                        in_=icps[:, 2 * s:2 * s + 2],
                                        axis=mybir.AxisListType.X, op=Alu.add)
        xg = gat.tile([128, 5, H], bf16)
        for s in range(4):
            nc.gpsimd.indirect_dma_start(
                out=xg[:, s, :], out_offset=None, in_=t["xN"][:],
                in_offset=bass.IndirectOffsetOnAxis(ap=icolT[:, s:s + 1], axis=0),
                bounds_check=N - 1, oob_is_err=False)

        # ---------- phase 5b: fallback ranks + fb inversion + gather ----------
        scanD = rt.tile([8, 512], f16)
        nc.vector.tensor_tensor_scan(out=scanD[:], data0=drop8[:],
                                     data1=zzh[0:8, :1].to_broadcast([8, 512]),
                                     initial=0.0, op0=Alu.add, op1=Alu.add)
        offD = ps_r.tile([8, 1], f32, tag="rps")
        nc.tensor.matmul(offD[:], lhsT=TL8_sb[:], rhs=scanD[:, 511:512],
                         start=True, stop=True)
        f511 = rt.tile([8, 1], f32)
        nc.vector.tensor_scalar(out=f511[:], in0=offD[:],
                                scalar1=float(CAP) - 1.0, scalar2=None,
                                op0=Alu.add)
        fbtot = ps_r.tile([1, 1], f32, tag="rps")
        nc.tensor.matmul(fbtot[:], lhsT=on8_sb[:], rhs=scanD[:, 511:512],
                         start=True, stop=True)
        fbt1 = rt.tile([1, 1], f32)
        nc.vector.tensor_copy(fbt1[:], fbtot[:])
        fbs = rt.tile([8, 512], f16)
        nc.vector.tensor_scalar(out=fbs[:], in0=scanD[:], scalar1=f511[:, :1],
                                scalar2=float(NSLOT - 1), op0=Alu.add, op1=Alu.min)
        fbm = rt.tile([8, 512], f32)
        nc.vector.tensor_tensor(out=fbm[:], in0=fbs[:], in1=drop8[:], op=Alu.mult)
        ptDf = ps_t.tile([128, 32], f32, tag="tps")
        for ib in range(4):
            nc.tensor.transpose(ptDf[:, ib * 8:(ib + 1) * 8],
                                fbm[:, ib * 128:(ib + 1) * 128], ident[0:8, 0:8])
        for c in range(8):
            for ib in range(4):
                b = c * 4 + ib
                Dbf = rt.tile([128, 128], f16, tag="Dbf", name=f"Dbf{b}", bufs=4)
                nc.vector.tensor_scalar(out=Dbf[:], in0=iota_sb[:, 512:640],
                                        scalar1=ptDf[:, ib * 8 + c:ib * 8 + c + 1],
                                        scalar2=None, op0=Alu.is_equal)
                nc.tensor.matmul(icps[:, 8:10], lhsT=Dbf[:],
                                 rhs=tokc_sb[:, 2 * b:2 * b + 2],
                                 start=False, stop=(b == 31),
                                 skip_group_check=True)
        with nc.allow_low_precision(reason="token ids are exact ints < 4096"):
            nc.vector.tensor_reduce(out=icolT[:, 4:5], in_=icps[:, 8:10],
                                    axis=mybir.AxisListType.X, op=Alu.add)
        nc.gpsimd.indirect_dma_start(
            out=xg[:, 4, :], out_offset=None, in_=t["xN"][:],
            in_offset=bass.IndirectOffsetOnAxis(ap=icolT[:, 4:5], axis=0),
            bounds_check=N - 1, oob_is_err=False)

        # ---------- counts output (off critical path) ----------
        t2rs = rt.tile([128, 1], f32)
        nc.vector.tensor_reduce(out=t2rs[:], in_=take2[:],
                                axis=mybir.AxisListType.X, op=Alu.add)
        t2e = ps_r.tile([8, 1], f32, tag="rps")
        nc.tensor.matmul(t2e[:], lhsT=E2_sb[:], rhs=t2rs[:], start=True,
                         stop=True)
        cnt8 = rt.tile([8, 1], f32)
        nc.vector.tensor_tensor(out=cnt8[:], in0=used[:], in1=t2e[:], op=Alu.add)

        if _PHASES < 6:
            return
        # ---------- phase 6: transpose own-expert gathered rows ----------
        # fb block (j=4) is transposed after FFN1 so the PE never waits on the
        # late fb gather.
        xgT = wpool.tile([128, 8, CAP + FBC], bf16)

        def xpose_block(j):
            for half in range(2):
                pt = ps_m.tile([128, 4, 128], bf16, tag="mmps")
                for q in range(4):
                    hc = half * 4 + q
                    nc.tensor.transpose(pt[:, q, :],
                                        xg[:, j, hc * 128:(hc + 1) * 128],
                                        identb[:])
                if half == 0:
                    nc.vector.tensor_copy(
                        out=xgT[:, 0:4, j * 128:(j + 1) * 128], in_=pt[:])
                else:
                    nc.scalar.activation(
                        xgT[:, 4:8, j * 128:(j + 1) * 128], pt[:], Act.Identity)

        for j in range(4):
            xpose_block(j)

        if _PHASES < 7:
            return
        # ---------- phase 7: expert FFN layer 1 (h^T = gelu(w1^T x^T + b1)) ----
        hT = wpool.tile([128, F // 128, CAP], bf16)
        for m in range(F // 128):
            pm = m + W1PRE
            if pm < F // 128:
                w1tile = w1s.tile([128, 8, 128], bf16, tag="w1t", name=f"w1t{pm}")
                nc.sync.dma_start(w1tile[:], t["w1c"][pm])
                w1t[pm] = w1tile
            ps = ps_m.tile([128, CAP], f32, tag="mmps")
            for k in range(8):
                nc.tensor.matmul(ps[:], lhsT=w1t[m][:, k, :],
                                 rhs=xgT[:, k, 0:CAP], start=(k == 0), stop=(k == 7))
            nc.scalar.activation(hT[:, m, :], ps[:], Act.Gelu,
                                 bias=b1_sb[:, m:m + 1])

        if _PHASES < 8:
            return
        # ---------- phase 8: fb transpose + fallback FFN L1 ----------
        xpose_block(4)
        hfbT = wpool.tile([128, FSH // 128, FBC], bf16)
        for m in range(FSH // 128):
            ps = ps_m.tile([128, FBC], f32, tag="mmps")
            for k in range(8):
                nc.tensor.matmul(ps[:], lhsT=sw1_sb[:, k, m * 128:(m + 1) * 128],
                                 rhs=xgT[:, k, CAP:CAP + FBC], start=(k == 0),
                                 stop=(k == 7))
            nc.scalar.activation(hfbT[:, m, :], ps[:], Act.Gelu,
                                 bias=sb1_sb[:, m:m + 1])

        if _PHASES < 9:
            return
        # ---------- phase 9: FFN layer 2 + interleaved fallback L2 ----------
        for m in range(H // 128):
            pm = m + W2PRE
            if pm < H // 128:
                w2tile = w2s.tile([128, F // 128, 128], bf16, tag="w2t", name=f"w2t{pm}")
                nc.sync.dma_start(w2tile[:], t["w2c"][pm])
                w2t[pm] = w2tile
            ps = ps_m.tile([128, CAP], f32, tag="mmps")
            for k in range(F // 128):
                nc.tensor.matmul(ps[:], lhsT=w2t[m][:, k, :], rhs=hT[:, k, :],
                                 start=(k == 0), stop=(k == F // 128 - 1))
            yt = outp.tile([128, CAP], f32, tag="yt")
            nc.scalar.activation(yt[:], ps[:], Act.Identity, bias=b2_sb[:, m:m + 1])
            nc.sync.dma_start(t["yT"][m * 128:(m + 1) * 128, :], yt[:])
            psf = ps_m.tile([128, FBC], f32, tag="fbps", bufs=1)
            for k in range(FSH // 128):
                nc.tensor.matmul(psf[:], lhsT=sw2_sb[:, k, m * 128:(m + 1) * 128],
                                 rhs=hfbT[:, k, :], start=(k == 0),
                                 stop=(k == FSH // 128 - 1))
            ft = outp.tile([128, FBC], f32, tag="ft")
            nc.scalar.activation(ft[:], psf[:], Act.Identity, bias=sb2_sb[:, m:m + 1])
            nc.sync.dma_start(t["fbT"][m * 128:(m + 1) * 128, :], ft[:])

        # ---------- idx/cnt outputs (end of SP stream; off critical path) ----
        idxo_pc = t["idxo"][:, None].rearrange("(s p) 1 -> s p 1", s=5)
        for s in range(5):
            nc.sync.dma_start(idxo_pc[s, :, :], icolT[:, s:s + 1])
        nc.sync.dma_start(t["cnt"][0:8, :], cnt8[:])
        nc.sync.dma_start(t["cnt"][8:9, :], fbt1[:])


def _get_nc(debug=False):
    key = ("ncdbg" if debug else "nc")
    if key not in _CACHE:
        _CACHE[key] = _build(debug)
    return _CACHE[key]


def _wt_layout(w):
    """[K, M] -> [M/128, 128, K/128, 128] with element [m, p, ko, mm] =
    w[ko*128 + p, m*128 + mm]; per-m-tile lhsT loads become contiguous."""
    K, M = w.shape
    return np.ascontiguousarray(
        w.reshape(K // 128, 128, M // 128, 128).transpose(2, 1, 0, 3))


def _col_layout(v, parts=128):
    """[D] vector -> [128, D//128] with element [p, m] = v[m*128 + p]."""
    return np.ascontiguousarray(v.reshape(-1, parts).T)


def make_in_maps(x, rw, rb, w1, b1, w2, b2, sw1, sb1, sw2, sb2):
    import ml_dtypes
    bf16 = ml_dtypes.bfloat16
    f16 = np.float16
    xf = np.ascontiguousarray(x.reshape(N, H).astype(np.float32))
    xT = np.ascontiguousarray(xf.T)
    xfb = np.ascontiguousarray(xf.astype(bf16))
    rwT = np.ascontiguousarray(rw.astype(np.float32).T)
    rbrow = np.ascontiguousarray(rb.astype(np.float32).reshape(1, E))
    ones1 = np.ones((1, 128), np.float32)

    T1 = np.zeros((128, 128), np.float32)
    E1 = np.zeros((128, 8), np.float32)
    BU = np.zeros((8, 128), np.float32)
    A1 = np.zeros((128, 8), np.float32)
    OV = np.zeros((8, 128), np.float32)
    AD1 = np.zeros((128, 8), np.float32)
    AD2 = np.zeros((128, 8), np.float32)
    A2 = np.zeros((128, 8), np.float32)
    E2 = np.zeros((128, 8), np.float32)
    for c in range(8):
        for m in range(16):
            for c2 in range(c):
                T1[c2 * 16 + m, c * 16 + m] = 1.0
        for e in range(8):
            E1[c * 16 + e, e] = 1.0
            BU[e, c * 16 + 8 + e] = 1.0
            A1[c * 16 + e, c] = 1.0
            OV[c, c * 16 + 8 + e] = 1.0
            AD1[c * 16 + e, c] = 1.0
            AD2[c * 16 + 8 + e, c] = 1.0
            A2[c * 16 + 8 + e, c] = 1.0
            E2[c * 16 + 8 + e, e] = 1.0
    TL8 = np.triu(np.ones((8, 8), np.float32), 1)
    on8 = np.ones((8, 1), np.float32)

    IOTA = np.tile(np.arange(642, dtype=np.float16), (128, 1))
    TOKC = np.zeros((128, 64), np.float16)
    for c in range(8):
        for ib in range(4):
            b = c * 4 + ib
            TOKC[:, 2 * b] = (c * 4 + ib) * 128.0
            TOKC[:, 2 * b + 1] = np.arange(128, dtype=np.float16)

    maps = []
    for k in range(NCORES):
        base = np.full(8, 3000.0, np.float32)
        base[k] = 0.0
        bm1 = np.zeros((128, 1), np.float32)
        for c in range(8):
            for e in range(8):
                bm1[c * 16 + e, 0] = base[e] - 1.0
                bm1[c * 16 + 8 + e, 0] = base[e] - 1.0
        maps.append({
            "xTc": np.ascontiguousarray(xT[:, k * NCH:(k + 1) * NCH]),
            "xN": xfb, "rwT": rwT, "rbrow": rbrow, "ones1": ones1,
            "T1": T1.astype(f16), "E1": E1.astype(f16), "BU": BU,
            "A1": np.ascontiguousarray(A1.astype(bf16)),
            "OV": np.ascontiguousarray(OV.astype(bf16)),
            "AD1": np.ascontiguousarray(AD1.astype(f16)),
            "AD2": np.ascontiguousarray(AD2.astype(f16)),
            "A2": np.ascontiguousarray(A2.astype(bf16)),
            "E2": E2, "TL8": TL8.astype(f16), "on8": on8.astype(f16),
            "bm1": np.ascontiguousarray(bm1), "IOTA": IOTA, "TOKC": TOKC,
            "w1c": _wt_layout(w1[k].astype(bf16)),
            "b1c": _col_layout(b1[k].astype(np.float32)),
            "w2c": _wt_layout(w2[k].astype(bf16)),
            "b2c": _col_layout(b2[k].astype(np.float32)),
            "sw1c": np.ascontiguousarray(sw1[:, k * FSH:(k + 1) * FSH].astype(bf16)),
            "sb1c": _col_layout(sb1[k * FSH:(k + 1) * FSH].astype(np.float32)),
            "sw2c": np.ascontiguousarray(sw2[k * FSH:(k + 1) * FSH, :].astype(bf16)),
            "sb2c": _col_layout((sb2 if k == 0 else
                                 np.zeros_like(sb2)).astype(np.float32)),
        })
    return maps


def assemble(results):
    """Combine per-core outputs into the full [B, T, H] output."""
    cnt0 = np.rint(np.asarray(results[0]["cnt"])).astype(np.int64).ravel()
    y = np.zeros((N, H), np.float32)
    for e in range(E):
        ne = int(min(cnt0[e], CAP))
        if ne <= 0:
            continue
        toks = np.asarray(results[e]["idxo"]).astype(np.int64)[:ne]
        y[toks] = np.asarray(results[e]["yT"])[:, :ne].T
    nfb = int(min(cnt0[E], FBC))
    if nfb > 0:
        toks = np.asarray(results[0]["idxo"]).astype(np.int64)[CAP:CAP + nfb]
        acc = np.zeros((H, nfb), np.float32)
        for k in range(NCORES):
            acc += np.asarray(results[k]["fbT"])[:, :nfb]
        y[toks] = acc.T
    return y.reshape(B, T, H)


def kernel(x, rw, rb, w1, b1, w2, b2, sw1, sb1, sw2, sb2):
    from concourse.bass_utils import run_bass_kernel_spmd
    args = [np.asarray(a) for a in
            (x, rw, rb, w1, b1, w2, b2, sw1, sb1, sw2, sb2)]
    nc = _get_nc()
    in_maps = make_in_maps(*args)
    res = run_bass_kernel_spmd(nc, in_maps, core_ids=list(range(NCORES)))
    return assemble(res.results)
